# revision 8
# baseline (speedup 1.0000x reference)
"""Trainium2 Bass kernel: causal self-attention (B=2, T=2048, D=1024, H=16).

NOTE: the reference's window constraint `(key - query) < 16` is vacuous under
causality, so this is FULL causal attention over T=2048 per batch.

Sharding: 8 cores = 2 batches x 4 head-groups (4 heads each). Every core
runs the identical program on (its batch's x^T, its group's W columns):
  - Q^T/K^T [256e, 2048t] and V [2048t, 256e] projections (bf16 inputs,
    fp32 accumulation)
  - blocked causal attention per head: 256-query chunks against 128-key
    blocks; exp on ScalarE without max-subtraction (scores are O(1));
    softmax denominator via an appended ones-column in V (row 64 of the
    PV matmul); only the two diagonal key blocks need masking
  - partial output projection y_g = Ot_g^T @ Wo_g^T  [2048, 1024]
The host sums the 4 per-group partial y's of each batch (no device
collectives) and stacks the 2 batches.

Matmuls run as float32r (1 PE cycle/row for moving dim >= 256).
"""

import numpy as np

# Problem shapes (hardcoded; kernel.py must be self-contained)
B, T, D = 2, 2048, 1024
H, HD = 16, 64
NCORES = 8
NG = 4                       # head groups
HG = H // NG                 # 4 heads per group
EG = HG * HD                 # 256 embedding cols per group
P = 128
CS = D // P                  # 8 contraction subtiles for Q/K/V projections
QCH = 256                    # query chunk
NQC = T // QCH               # 8 query chunks
NKB = T // P                 # 16 key blocks

MM_DTYPE = "float32r"        # S/U/O matmul operand dtype: float32r | float32

_nc_cache = {}


def _build_nc(mm_dtype_name):
    import concourse.bacc as bacc
    import concourse.tile as tile
    import concourse.mybir as mybir

    f32 = mybir.dt.float32
    bf16 = mybir.dt.bfloat16
    mmdt = getattr(mybir.dt, mm_dtype_name)
    Exp = mybir.ActivationFunctionType.Exp

    nc = bacc.Bacc("TRN2", target_bir_lowering=False, debug=False)

    xt_d = nc.dram_tensor("xt", [D, T], bf16, kind="ExternalInput")
    wq_d = nc.dram_tensor("wqt", [D, EG], bf16, kind="ExternalInput")
    wk_d = nc.dram_tensor("wkt", [D, EG], bf16, kind="ExternalInput")
    wv_d = nc.dram_tensor("wvt", [D, EG], bf16, kind="ExternalInput")
    wo_d = nc.dram_tensor("wot", [EG, D], bf16, kind="ExternalInput")
    md_d = nc.dram_tensor("maskd", [P, 2, QCH], bf16, kind="ExternalInput")
    y_d = nc.dram_tensor("y", [T, D], f32, kind="ExternalOutput")

    def mm(out, lhsT, rhs, **kw):
        nc.tensor.matmul(out, lhsT, rhs, **kw)

    with tile.TileContext(nc) as tc:
        with (
            tc.tile_pool(name="const", bufs=1) as constp,
            tc.tile_pool(name="big", bufs=1) as bigp,
            tc.tile_pool(name="ebuf", bufs=2) as ebufp,
            tc.tile_pool(name="work", bufs=3) as workp,
            tc.tile_pool(name="zs", bufs=3) as zp,
            tc.tile_pool(name="pp", bufs=2, space="PSUM") as pp,
            tc.tile_pool(name="sp", bufs=4, space="PSUM") as sp,
            tc.tile_pool(name="up", bufs=2, space="PSUM") as up,
        ):
            # ---- load x^T (c on partitions), W slices, diagonal masks ----
            xt = bigp.tile([P, CS, T], bf16, name="xt")
            xt_r = xt_d[:].rearrange("(s p) t -> p s t", p=P)
            for s in range(CS):
                nc.sync.dma_start(xt[:, s, :], xt_r[:, s, :])

            wq = constp.tile([P, CS, EG], bf16, name="wq")
            wk = constp.tile([P, CS, EG], bf16, name="wk")
            wv = constp.tile([P, CS, EG], bf16, name="wv")
            for w_sb, w_dr in ((wq, wq_d), (wk, wk_d), (wv, wv_d)):
                nc.sync.dma_start(w_sb, w_dr[:].rearrange("(s p) e -> p s e", p=P))
            wo = constp.tile([P, 2, D], bf16, name="wo")
            nc.sync.dma_start(wo, wo_d[:].rearrange("(s p) e -> p s e", p=P))
            md = constp.tile([P, 2, QCH], bf16, name="md")
            nc.sync.dma_start(md, md_d[:])

            # ---- Q^T / K^T projections: [e_local on partitions, t free] ----
            qt = bigp.tile([P, 2, T], bf16, name="qt")
            kt = bigp.tile([P, 2, T], bf16, name="kt")
            for dst, w_sb in ((kt, wk), (qt, wq)):
                for et in range(2):
                    for tc4 in range(4):
                        ps = pp.tile([P, 512], f32, tag="proj", name="ps_p")
                        for s in range(CS):
                            nc.tensor.matmul(
                                ps, w_sb[:, s, et * P:(et + 1) * P],
                                xt[:, s, tc4 * 512:(tc4 + 1) * 512],
                                start=(s == 0), stop=(s == CS - 1))
                        nc.vector.tensor_copy(
                            out=dst[:, et, tc4 * 512:(tc4 + 1) * 512], in_=ps)

            # ---- V: [t on partitions, head, 64+1] with ones column ----
            vt = bigp.tile([P, NKB, HG, HD + 1], bf16, name="vt")
            nc.vector.memset(vt[:, :, :, HD:HD + 1], 1.0)
            for kb in range(NKB):
                ps = pp.tile([P, 512], f32, tag="proj", name="ps_v")
                for s in range(CS):
                    nc.tensor.matmul(ps[:, :EG], xt[:, s, kb * P:(kb + 1) * P],
                                     wv[:, s, :],
                                     start=(s == 0), stop=(s == CS - 1))
                nc.vector.tensor_copy(
                    out=vt[:, kb, :, 0:HD],
                    in_=ps[:, :EG].rearrange("p (j d) -> p j d", d=HD))

            # ---- blocked causal attention ----
            # Per head h (local), query chunk qc: key blocks 0..2qc+1.
            # Key blocks come in pairs sharing one PSUM bank; the final pair
            # (diagonal) is masked after exp. Head pairs (2a, 2a+1) are
            # emitted adjacently so their S matmuls (row groups 0-63 /
            # 64-127) can overlap in the PE array.
            ot = bigp.tile([P, 2, T], bf16, name="ot")

            def attn(qc, a):
                # heads 2a, 2a+1 for query chunk qc
                qs = qc * QCH
                nbp = qc + 1          # key-block pairs
                es = []
                for hh in (0, 1):
                    e = ebufp.tile([P, NQC, 2 * QCH], bf16, tag="ebuf",
                                   name="ebuf")
                    es.append(e)
                for kbp in range(nbp):
                    ss = []
                    for hh in (0, 1):
                        s01 = sp.tile([P, 2 * QCH], f32, tag="s01", name="s01")
                        ss.append(s01)
                    for hh in (0, 1):
                        po = 64 * hh
                        qsl = qt[po:po + 64, a, qs:qs + QCH]
                        mm(ss[hh][:, 0:QCH],
                           kt[po:po + 64, a, 2 * kbp * P:(2 * kbp + 1) * P],
                           qsl, start=True, stop=False, skip_group_check=True)
                    for hh in (0, 1):
                        po = 64 * hh
                        qsl = qt[po:po + 64, a, qs:qs + QCH]
                        mm(ss[hh][:, QCH:2 * QCH],
                           kt[po:po + 64, a, (2 * kbp + 1) * P:(2 * kbp + 2) * P],
                           qsl, start=False, stop=True, skip_group_check=True)
                    for hh in (0, 1):
                        nc.scalar.activation(out=es[hh][:, kbp, :], in_=ss[hh],
                                             func=Exp, scale=0.125)
                # mask the diagonal pair (last kb pair: blocks 2qc, 2qc+1)
                for hh in (0, 1):
                    nc.vector.tensor_mul(
                        es[hh][:, qc, :], es[hh][:, qc, :],
                        md[:].rearrange("p m q -> p (m q)"))
                for hh in (0, 1):
                    h = 2 * a + hh
                    po = 64 * hh
                    u = up.tile([HD + 1, QCH], f32, tag="u", name="u")
                    for kb in range(2 * qc + 2):
                        mm(u, vt[:, kb, h, :],
                           es[hh][:, kb // 2, (kb % 2) * QCH:(kb % 2 + 1) * QCH],
                           start=(kb == 0), stop=(kb == 2 * qc + 1))
                    zr = zp.tile([1, QCH], f32, tag="zr", name="zr")
                    nc.vector.reciprocal(zr, u[HD:HD + 1, :])
                    zb = zp.tile([HD, QCH], f32, tag="zb", name="zb")
                    nc.gpsimd.partition_broadcast(zb, zr)
                    nc.vector.tensor_mul(ot[po:po + 64, a, qs:qs + QCH],
                                         u[0:HD, :], zb)

            for qc in range(NQC):
                for a in range(2):
                    attn(qc, a)

            # ---- partial output projection: y_g = Ot_g^T @ Wo_g^T ----
            for tc16 in range(16):
                for eh in range(2):
                    ps = pp.tile([P, 512], f32, tag="proj", name="ps_o")
                    for s in range(2):
                        mm(ps, ot[:, s, tc16 * P:(tc16 + 1) * P],
                           wo[:, s, eh * 512:(eh + 1) * 512],
                           start=(s == 0), stop=(s == 1))
                    ysb = workp.tile([P, 512], f32, tag="ysb", name="ysb")
                    nc.vector.tensor_copy(out=ysb, in_=ps)
                    nc.sync.dma_start(
                        y_d[:][tc16 * P:(tc16 + 1) * P,
                               eh * 512:(eh + 1) * 512], ysb)

    nc.compile()
    return nc


def get_nc(mm_dtype_name=MM_DTYPE):
    if mm_dtype_name not in _nc_cache:
        _nc_cache[mm_dtype_name] = _build_nc(mm_dtype_name)
    return _nc_cache[mm_dtype_name]


def _diag_masks():
    jk = np.arange(P)[:, None]
    i = np.arange(QCH)[None, :]
    md = np.empty((P, 2, QCH), np.float32)
    md[:, 0, :] = (jk <= i)          # key block 2qc vs queries qs..qs+255
    md[:, 1, :] = (jk <= i - P)      # key block 2qc+1
    return md


def prep_inputs(x, Wq, Wk, Wv, Wo):
    import ml_dtypes
    bf16 = ml_dtypes.bfloat16
    x = np.asarray(x, np.float32)
    md = _diag_masks()
    wqt = np.asarray(Wq, np.float32).T    # [c, e]
    wkt = np.asarray(Wk, np.float32).T
    wvt = np.asarray(Wv, np.float32).T
    wot = np.asarray(Wo, np.float32).T    # [c, e]

    in_maps = []
    for c in range(NCORES):
        b, g = divmod(c, NG)
        e0 = g * EG
        in_maps.append({
            "xt": np.ascontiguousarray(x[b].T).astype(bf16),
            "wqt": np.ascontiguousarray(wqt[:, e0:e0 + EG]).astype(bf16),
            "wkt": np.ascontiguousarray(wkt[:, e0:e0 + EG]).astype(bf16),
            "wvt": np.ascontiguousarray(wvt[:, e0:e0 + EG]).astype(bf16),
            "wot": np.ascontiguousarray(wot[e0:e0 + EG, :]).astype(bf16),
            "maskd": md.astype(bf16),
        })
    return in_maps


def gather_output(results):
    ys = [np.asarray(r["y"], np.float32) for r in results]
    out = np.empty((B, T, D), np.float32)
    for b in range(B):
        out[b] = sum(ys[b * NG:(b + 1) * NG])
    return out


last_run = {}


def kernel(x, Wq, Wk, Wv, Wo, _trace=False):
    from concourse.bass_utils import run_bass_kernel_spmd

    nc = get_nc()
    in_maps = prep_inputs(x, Wq, Wk, Wv, Wo)
    res = run_bass_kernel_spmd(
        nc, in_maps, core_ids=list(range(NCORES)), trace=_trace)
    last_run["results"] = res
    return gather_output(res.results)


# revision 10
# speedup vs baseline: 3.5765x; 3.5765x over previous
"""Trainium2 Bass kernel: causal self-attention (B=2, T=2048, D=1024, H=16).

NOTE: the reference's window constraint `(key - query) < 16` is vacuous under
causality, so this is FULL causal attention over T=2048 per batch.

Sharding: 8 cores = 2 batches x 4 head-groups (4 heads each). Every core
runs the identical program on (its batch's x^T, its group's W columns):
  - Q^T/K^T [256e, 2048t] and V [2048t, 256e] projections (bf16 inputs,
    fp32 accumulation)
  - blocked causal attention per head: 256-query chunks against 128-key
    blocks; exp on ScalarE without max-subtraction (scores are O(1));
    softmax denominator via an appended ones-column in V (row 64 of the
    PV matmul); only the two diagonal key blocks need masking
  - partial output projection y_g = Ot_g^T @ Wo_g^T  [2048, 1024]
The host sums the 4 per-group partial y's of each batch (no device
collectives) and stacks the 2 batches.

Matmul operands are bf16 (1 PE cycle/row, fp32 PSUM accumulation).

`loop_reps > 0` builds a timing variant with the whole body inside a
hardware For_i loop (used by bench.py to measure per-execution HW time
despite multi-ms dispatch jitter).
"""

import numpy as np

# Problem shapes (hardcoded; kernel.py must be self-contained)
B, T, D = 2, 2048, 1024
H, HD = 16, 64
NCORES = 8
NG = 4                       # head groups
HG = H // NG                 # 4 heads per group
EG = HG * HD                 # 256 embedding cols per group
P = 128
CS = D // P                  # 8 contraction subtiles for Q/K/V projections
QCH = 256                    # query chunk
NQC = T // QCH               # 8 query chunks
NKB = T // P                 # 16 key blocks

_nc_cache = {}


def _emit_body(nc, env):
    """Emit one full forward pass (projections + attention + out-proj)."""
    import concourse.mybir as mybir

    f32 = mybir.dt.float32
    bf16 = mybir.dt.bfloat16
    Exp = mybir.ActivationFunctionType.Exp
    xt, wq, wk, wv, wo, md, y_d = (env[k] for k in
                                   ("xt", "wq", "wk", "wv", "wo", "md", "y_d"))
    bigp, ebufp, workp, zp, pp, sp, up = (env[k] for k in
                                          ("bigp", "ebufp", "workp", "zp",
                                           "pp", "sp", "up"))
    mm = nc.tensor.matmul

    # ---- Q^T / K^T projections: [e_local on partitions, t free] ----
    qt = bigp.tile([P, 2, T], bf16, tag="qt", name="qt")
    kt = bigp.tile([P, 2, T], bf16, tag="kt", name="kt")
    for dst, w_sb in ((kt, wk), (qt, wq)):
        for et in range(2):
            for tc4 in range(4):
                ps = pp.tile([P, 512], f32, tag="proj", name="ps_p")
                for s in range(CS):
                    mm(ps, w_sb[:, s, et * P:(et + 1) * P],
                       xt[:, s, tc4 * 512:(tc4 + 1) * 512],
                       start=(s == 0), stop=(s == CS - 1))
                nc.vector.tensor_copy(
                    out=dst[:, et, tc4 * 512:(tc4 + 1) * 512], in_=ps)

    # ---- V: [t on partitions, head, 64+1] with ones column ----
    vt = bigp.tile([P, NKB, HG, HD + 1], bf16, tag="vt", name="vt")
    nc.vector.memset(vt[:, :, :, HD:HD + 1], 1.0)
    for kb in range(NKB):
        ps = pp.tile([P, 512], f32, tag="proj", name="ps_v")
        for s in range(CS):
            mm(ps[:, :EG], xt[:, s, kb * P:(kb + 1) * P], wv[:, s, :],
               start=(s == 0), stop=(s == CS - 1))
        nc.vector.tensor_copy(
            out=vt[:, kb, :, 0:HD],
            in_=ps[:, :EG].rearrange("p (j d) -> p j d", d=HD))

    # ---- blocked causal attention ----
    # Per head, query chunk qc uses key blocks 0..2qc+1. Key blocks come
    # in pairs sharing one PSUM bank; only the final (diagonal) pair is
    # masked after exp. Head pairs (2a, 2a+1) are emitted adjacently so
    # their S matmuls (row groups 0-63 / 64-127) can overlap in the PE.
    ot = bigp.tile([P, 2, T], bf16, tag="ot", name="ot")

    def attn(qc, a):
        qs = qc * QCH
        es = []
        for hh in (0, 1):
            e = ebufp.tile([P, NQC, 2 * QCH], bf16, tag="ebuf", name="ebuf")
            es.append(e)
        for kbp in range(qc + 1):
            ss = [sp.tile([P, 2 * QCH], f32, tag="s01", name="s01")
                  for hh in (0, 1)]
            for half in (0, 1):
                for hh in (0, 1):
                    po = 64 * hh
                    qsl = qt[po:po + 64, a, qs:qs + QCH]
                    kb = 2 * kbp + half
                    mm(ss[hh][:, half * QCH:(half + 1) * QCH],
                       kt[po:po + 64, a, kb * P:(kb + 1) * P], qsl,
                       start=(half == 0), stop=(half == 1),
                       skip_group_check=True)
            for hh in (0, 1):
                nc.scalar.activation(out=es[hh][:, kbp, :], in_=ss[hh],
                                     func=Exp, scale=0.125)
        # mask the diagonal pair (pair index qc = blocks 2qc, 2qc+1)
        for hh in (0, 1):
            nc.vector.tensor_mul(
                es[hh][:, qc, :], es[hh][:, qc, :],
                md[:].rearrange("p m q -> p (m q)"))
        for hh in (0, 1):
            h = 2 * a + hh
            po = 64 * hh
            u = up.tile([HD + 1, QCH], f32, tag="u", name="u")
            for kb in range(2 * qc + 2):
                mm(u, vt[:, kb, h, :],
                   es[hh][:, kb // 2, (kb % 2) * QCH:(kb % 2 + 1) * QCH],
                   start=(kb == 0), stop=(kb == 2 * qc + 1))
            zr = zp.tile([1, QCH], f32, tag="zr", name="zr")
            nc.vector.reciprocal(zr, u[HD:HD + 1, :])
            zb = zp.tile([HD, QCH], f32, tag="zb", name="zb")
            nc.gpsimd.partition_broadcast(zb, zr)
            nc.vector.tensor_mul(ot[po:po + 64, a, qs:qs + QCH],
                                 u[0:HD, :], zb)

    for qc in range(NQC):
        for a in range(2):
            attn(qc, a)

    # ---- partial output projection: y_g = Ot_g^T @ Wo_g^T ----
    for tc16 in range(16):
        for eh in range(2):
            ps = pp.tile([P, 512], f32, tag="proj", name="ps_o")
            for s in range(2):
                mm(ps, ot[:, s, tc16 * P:(tc16 + 1) * P],
                   wo[:, s, eh * 512:(eh + 1) * 512],
                   start=(s == 0), stop=(s == 1))
            ysb = workp.tile([P, 512], f32, tag="ysb", name="ysb")
            nc.vector.tensor_copy(out=ysb, in_=ps)
            nc.sync.dma_start(
                y_d[:][tc16 * P:(tc16 + 1) * P, eh * 512:(eh + 1) * 512], ysb)


def _build_nc(loop_reps=0):
    from contextlib import nullcontext

    import concourse.bacc as bacc
    import concourse.tile as tile
    import concourse.mybir as mybir

    f32 = mybir.dt.float32
    bf16 = mybir.dt.bfloat16

    nc = bacc.Bacc("TRN2", target_bir_lowering=False, debug=False)

    xt_d = nc.dram_tensor("xt", [D, T], bf16, kind="ExternalInput")
    wq_d = nc.dram_tensor("wqt", [D, EG], bf16, kind="ExternalInput")
    wk_d = nc.dram_tensor("wkt", [D, EG], bf16, kind="ExternalInput")
    wv_d = nc.dram_tensor("wvt", [D, EG], bf16, kind="ExternalInput")
    wo_d = nc.dram_tensor("wot", [EG, D], bf16, kind="ExternalInput")
    md_d = nc.dram_tensor("maskd", [P, 2, QCH], bf16, kind="ExternalInput")
    y_d = nc.dram_tensor("y", [T, D], f32, kind="ExternalOutput")

    with tile.TileContext(nc) as tc:
        with (
            tc.tile_pool(name="const", bufs=1) as constp,
            tc.tile_pool(name="big", bufs=1) as bigp,
            tc.tile_pool(name="ebuf", bufs=2) as ebufp,
            tc.tile_pool(name="work", bufs=3) as workp,
            tc.tile_pool(name="zs", bufs=3) as zp,
            tc.tile_pool(name="pp", bufs=2, space="PSUM") as pp,
            tc.tile_pool(name="sp", bufs=4, space="PSUM") as sp,
            tc.tile_pool(name="up", bufs=2, space="PSUM") as up,
        ):
            # ---- load x^T (c on partitions), W slices, diagonal masks ----
            xt = bigp.tile([P, CS, T], bf16, name="xt")
            xt_r = xt_d[:].rearrange("(s p) t -> p s t", p=P)
            for s in range(CS):
                nc.sync.dma_start(xt[:, s, :], xt_r[:, s, :])

            wq = constp.tile([P, CS, EG], bf16, name="wq")
            wk = constp.tile([P, CS, EG], bf16, name="wk")
            wv = constp.tile([P, CS, EG], bf16, name="wv")
            for w_sb, w_dr in ((wq, wq_d), (wk, wk_d), (wv, wv_d)):
                nc.sync.dma_start(w_sb,
                                  w_dr[:].rearrange("(s p) e -> p s e", p=P))
            wo = constp.tile([P, 2, D], bf16, name="wo")
            nc.sync.dma_start(wo, wo_d[:].rearrange("(s p) e -> p s e", p=P))
            md = constp.tile([P, 2, QCH], bf16, name="md")
            nc.sync.dma_start(md, md_d[:])

            env = dict(xt=xt, wq=wq, wk=wk, wv=wv, wo=wo, md=md, y_d=y_d,
                       bigp=bigp, ebufp=ebufp, workp=workp, zp=zp,
                       pp=pp, sp=sp, up=up)
            if loop_reps:
                with tc.For_i(0, loop_reps, 1):
                    _emit_body(nc, env)
            else:
                _emit_body(nc, env)

    nc.compile()
    return nc


def get_nc(loop_reps=0):
    if loop_reps not in _nc_cache:
        _nc_cache[loop_reps] = _build_nc(loop_reps)
    return _nc_cache[loop_reps]


def _diag_masks():
    jk = np.arange(P)[:, None]
    i = np.arange(QCH)[None, :]
    md = np.empty((P, 2, QCH), np.float32)
    md[:, 0, :] = (jk <= i)          # key block 2qc vs queries qs..qs+255
    md[:, 1, :] = (jk <= i - P)      # key block 2qc+1
    return md


def prep_inputs(x, Wq, Wk, Wv, Wo):
    import ml_dtypes
    bf16 = ml_dtypes.bfloat16
    x = np.asarray(x, np.float32)
    md = _diag_masks()
    wqt = np.asarray(Wq, np.float32).T    # [c, e]
    wkt = np.asarray(Wk, np.float32).T
    wvt = np.asarray(Wv, np.float32).T
    wot = np.asarray(Wo, np.float32).T    # [c, e]

    in_maps = []
    for c in range(NCORES):
        b, g = divmod(c, NG)
        e0 = g * EG
        in_maps.append({
            "xt": np.ascontiguousarray(x[b].T).astype(bf16),
            "wqt": np.ascontiguousarray(wqt[:, e0:e0 + EG]).astype(bf16),
            "wkt": np.ascontiguousarray(wkt[:, e0:e0 + EG]).astype(bf16),
            "wvt": np.ascontiguousarray(wvt[:, e0:e0 + EG]).astype(bf16),
            "wot": np.ascontiguousarray(wot[e0:e0 + EG, :]).astype(bf16),
            "maskd": md.astype(bf16),
        })
    return in_maps


def gather_output(results):
    ys = [np.asarray(r["y"], np.float32) for r in results]
    out = np.empty((B, T, D), np.float32)
    for b in range(B):
        out[b] = sum(ys[b * NG:(b + 1) * NG])
    return out


last_run = {}


def kernel(x, Wq, Wk, Wv, Wo, _trace=False):
    from concourse.bass_utils import run_bass_kernel_spmd

    nc = get_nc()
    in_maps = prep_inputs(x, Wq, Wk, Wv, Wo)
    res = run_bass_kernel_spmd(
        nc, in_maps, core_ids=list(range(NCORES)), trace=_trace)
    last_run["results"] = res
    return gather_output(res.results)


# revision 12
# speedup vs baseline: 4.4991x; 1.2580x over previous
"""Trainium2 Bass kernel: causal self-attention (B=2, T=2048, D=1024, H=16).

NOTE: the reference's window constraint `(key - query) < 16` is vacuous under
causality, so this is FULL causal attention over T=2048 per batch.

Sharding: 8 cores = 2 batches x 4 head-groups (4 heads each). Every core
runs the identical program on (its batch's x^T, its group's W columns):
  - Q^T/K^T [256e, 2048t] and V [2048t, 256e] projections (bf16 inputs,
    fp32 accumulation)
  - blocked causal attention per head: 256-query chunks against 128-key
    blocks; exp on ScalarE without max-subtraction (scores are O(1));
    softmax denominator via an appended ones-column in V (row 64 of the
    PV matmul); only the two diagonal key blocks need masking
  - partial output projection y_g = Ot_g^T @ Wo_g^T  [2048, 1024]
The host sums the 4 per-group partial y's of each batch (no device
collectives) and stacks the 2 batches.

Matmul operands are bf16 (1 PE cycle/row, fp32 PSUM accumulation).

`loop_reps > 0` builds a timing variant with the whole body inside a
hardware For_i loop (used by bench.py to measure per-execution HW time
despite multi-ms dispatch jitter).
"""

import numpy as np

# Problem shapes (hardcoded; kernel.py must be self-contained)
B, T, D = 2, 2048, 1024
H, HD = 16, 64
NCORES = 8
NG = 4                       # head groups
HG = H // NG                 # 4 heads per group
EG = HG * HD                 # 256 embedding cols per group
P = 128
CS = D // P                  # 8 contraction subtiles for Q/K/V projections
QCH = 256                    # query chunk
NQC = T // QCH               # 8 query chunks
NKB = T // P                 # 16 key blocks

_nc_cache = {}


def _emit_body(nc, env):
    """Emit one full forward pass (projections + attention + out-proj)."""
    import concourse.mybir as mybir

    f32 = mybir.dt.float32
    bf16 = mybir.dt.bfloat16
    Exp = mybir.ActivationFunctionType.Exp
    xt, wq, wk, wv, wo, md, y_d = (env[k] for k in
                                   ("xt", "wq", "wk", "wv", "wo", "md", "y_d"))
    bigp, ebufp, workp, zp, pp, sp, up = (env[k] for k in
                                          ("bigp", "ebufp", "workp", "zp",
                                           "pp", "sp", "up"))
    mm = nc.tensor.matmul

    # ---- Q^T / K^T projections: [e_local on partitions, t free] ----
    qt = bigp.tile([P, 2, T], bf16, tag="qt", name="qt")
    kt = bigp.tile([P, 2, T], bf16, tag="kt", name="kt")
    for dst, w_sb in ((kt, wk), (qt, wq)):
        for et in range(2):
            for tc4 in range(4):
                ps = pp.tile([P, 512], f32, tag="proj", name="ps_p")
                for s in range(CS):
                    mm(ps, w_sb[:, s, et * P:(et + 1) * P],
                       xt[:, s, tc4 * 512:(tc4 + 1) * 512],
                       start=(s == 0), stop=(s == CS - 1))
                nc.vector.tensor_copy(
                    out=dst[:, et, tc4 * 512:(tc4 + 1) * 512], in_=ps)

    # ---- V: [t on partitions, head, 64+1] with ones column ----
    vt = bigp.tile([P, NKB, HG, HD + 1], bf16, tag="vt", name="vt")
    nc.vector.memset(vt[:, :, :, HD:HD + 1], 1.0)
    for kb in range(NKB):
        ps = pp.tile([P, 512], f32, tag="proj", name="ps_v")
        for s in range(CS):
            mm(ps[:, :EG], xt[:, s, kb * P:(kb + 1) * P], wv[:, s, :],
               start=(s == 0), stop=(s == CS - 1))
        nc.vector.tensor_copy(
            out=vt[:, kb, :, 0:HD],
            in_=ps[:, :EG].rearrange("p (j d) -> p j d", d=HD))

    # ---- blocked causal attention ----
    # Per head, query chunk qc uses key blocks 0..2qc+1. Key blocks come
    # in pairs sharing one PSUM bank; only the final (diagonal) pair is
    # masked after exp. Head pairs (2a, 2a+1) are emitted adjacently so
    # their S matmuls (row groups 0-63 / 64-127) can overlap in the PE.
    ot = bigp.tile([P, 2, T], bf16, tag="ot", name="ot")
    estate = {}

    def attn_s(it):
        """S matmuls + exp (+ diagonal mask) for iteration (qc, a)."""
        qc, a = it
        qs = qc * QCH
        # E layout per head-pair: [k_part, kbp, hh, 2 key blocks, 256 q]
        e = ebufp.tile([P, NQC, 2, 2, QCH], bf16, tag="ebuf", name="ebuf")
        for kbp in range(qc + 1):
            # one 2-bank psum tile: bank0 = head 2a, bank1 = head 2a+1
            s4 = sp.tile([P, 2, 2 * QCH], f32, tag="s4", name="s4")
            for half in (0, 1):
                for hh in (0, 1):
                    po = 64 * hh
                    qsl = qt[po:po + 64, a, qs:qs + QCH]
                    kb = 2 * kbp + half
                    mm(s4[:, hh, half * QCH:(half + 1) * QCH],
                       kt[po:po + 64, a, kb * P:(kb + 1) * P], qsl,
                       start=(half == 0), stop=(half == 1),
                       skip_group_check=True)
            nc.scalar.activation(out=e[:, kbp, :, :, :], in_=s4,
                                 func=Exp, scale=0.125)
        # mask the diagonal pair (pair index qc = blocks 2qc, 2qc+1)
        nc.vector.tensor_mul(
            e[:, qc, :, :, :], e[:, qc, :, :, :],
            md[:, None].to_broadcast((P, 2, 2, QCH)))
        estate[it] = e

    def attn_u(it):
        """PV accumulation + normalization for iteration (qc, a)."""
        qc, a = it
        qs = qc * QCH
        e = estate.pop(it)
        for hh in (0, 1):
            h = 2 * a + hh
            po = 64 * hh
            u = up.tile([HD + 1, QCH], f32, tag="u", name="u")
            for kb in range(2 * qc + 2):
                mm(u, vt[:, kb, h, :], e[:, kb // 2, hh, kb % 2, :],
                   start=(kb == 0), stop=(kb == 2 * qc + 1))
            zr = zp.tile([1, QCH], f32, tag="zr", name="zr")
            nc.vector.reciprocal(zr, u[HD:HD + 1, :])
            zb = zp.tile([HD, QCH], f32, tag="zb", name="zb")
            nc.gpsimd.partition_broadcast(zb, zr)
            nc.vector.tensor_mul(ot[po:po + 64, a, qs:qs + QCH],
                                 u[0:HD, :], zb)

    iters = [(qc, a) for qc in range(NQC) for a in range(2)]
    LAG = 1
    for i, it in enumerate(iters):
        attn_s(it)
        if i >= LAG:
            attn_u(iters[i - LAG])
    for it in iters[len(iters) - LAG:]:
        attn_u(it)

    # ---- partial output projection: y_g = Ot_g^T @ Wo_g^T ----
    for tc16 in range(16):
        for eh in range(2):
            ps = pp.tile([P, 512], f32, tag="proj", name="ps_o")
            for s in range(2):
                mm(ps, ot[:, s, tc16 * P:(tc16 + 1) * P],
                   wo[:, s, eh * 512:(eh + 1) * 512],
                   start=(s == 0), stop=(s == 1))
            ysb = workp.tile([P, 512], f32, tag="ysb", name="ysb")
            nc.vector.tensor_copy(out=ysb, in_=ps)
            nc.sync.dma_start(
                y_d[:][tc16 * P:(tc16 + 1) * P, eh * 512:(eh + 1) * 512], ysb)


def _build_nc(loop_reps=0):
    from contextlib import nullcontext

    import concourse.bacc as bacc
    import concourse.tile as tile
    import concourse.mybir as mybir

    f32 = mybir.dt.float32
    bf16 = mybir.dt.bfloat16

    nc = bacc.Bacc("TRN2", target_bir_lowering=False, debug=False)

    xt_d = nc.dram_tensor("xt", [D, T], bf16, kind="ExternalInput")
    wq_d = nc.dram_tensor("wqt", [D, EG], bf16, kind="ExternalInput")
    wk_d = nc.dram_tensor("wkt", [D, EG], bf16, kind="ExternalInput")
    wv_d = nc.dram_tensor("wvt", [D, EG], bf16, kind="ExternalInput")
    wo_d = nc.dram_tensor("wot", [EG, D], bf16, kind="ExternalInput")
    md_d = nc.dram_tensor("maskd", [P, 2, QCH], bf16, kind="ExternalInput")
    y_d = nc.dram_tensor("y", [T, D], f32, kind="ExternalOutput")

    with tile.TileContext(nc) as tc:
        with (
            tc.tile_pool(name="const", bufs=1) as constp,
            tc.tile_pool(name="big", bufs=1) as bigp,
            tc.tile_pool(name="ebuf", bufs=2) as ebufp,
            tc.tile_pool(name="work", bufs=3) as workp,
            tc.tile_pool(name="zs", bufs=3) as zp,
            tc.tile_pool(name="pp", bufs=2, space="PSUM") as pp,
            tc.tile_pool(name="sp", bufs=2, space="PSUM") as sp,
            tc.tile_pool(name="up", bufs=2, space="PSUM") as up,
        ):
            # ---- load x^T (c on partitions), W slices, diagonal masks ----
            xt = bigp.tile([P, CS, T], bf16, name="xt")
            xt_r = xt_d[:].rearrange("(s p) t -> p s t", p=P)
            for s in range(CS):
                nc.sync.dma_start(xt[:, s, :], xt_r[:, s, :])

            wq = constp.tile([P, CS, EG], bf16, name="wq")
            wk = constp.tile([P, CS, EG], bf16, name="wk")
            wv = constp.tile([P, CS, EG], bf16, name="wv")
            for w_sb, w_dr in ((wq, wq_d), (wk, wk_d), (wv, wv_d)):
                nc.sync.dma_start(w_sb,
                                  w_dr[:].rearrange("(s p) e -> p s e", p=P))
            wo = constp.tile([P, 2, D], bf16, name="wo")
            nc.sync.dma_start(wo, wo_d[:].rearrange("(s p) e -> p s e", p=P))
            md = constp.tile([P, 2, QCH], bf16, name="md")
            nc.sync.dma_start(md, md_d[:])

            env = dict(xt=xt, wq=wq, wk=wk, wv=wv, wo=wo, md=md, y_d=y_d,
                       bigp=bigp, ebufp=ebufp, workp=workp, zp=zp,
                       pp=pp, sp=sp, up=up)
            if loop_reps:
                with tc.For_i(0, loop_reps, 1):
                    _emit_body(nc, env)
            else:
                _emit_body(nc, env)

    nc.compile()
    return nc


def get_nc(loop_reps=0):
    if loop_reps not in _nc_cache:
        _nc_cache[loop_reps] = _build_nc(loop_reps)
    return _nc_cache[loop_reps]


def _diag_masks():
    jk = np.arange(P)[:, None]
    i = np.arange(QCH)[None, :]
    md = np.empty((P, 2, QCH), np.float32)
    md[:, 0, :] = (jk <= i)          # key block 2qc vs queries qs..qs+255
    md[:, 1, :] = (jk <= i - P)      # key block 2qc+1
    return md


def prep_inputs(x, Wq, Wk, Wv, Wo):
    import ml_dtypes
    bf16 = ml_dtypes.bfloat16
    x = np.asarray(x, np.float32)
    md = _diag_masks()
    wqt = np.asarray(Wq, np.float32).T    # [c, e]
    wkt = np.asarray(Wk, np.float32).T
    wvt = np.asarray(Wv, np.float32).T
    wot = np.asarray(Wo, np.float32).T    # [c, e]

    in_maps = []
    for c in range(NCORES):
        b, g = divmod(c, NG)
        e0 = g * EG
        in_maps.append({
            "xt": np.ascontiguousarray(x[b].T).astype(bf16),
            "wqt": np.ascontiguousarray(wqt[:, e0:e0 + EG]).astype(bf16),
            "wkt": np.ascontiguousarray(wkt[:, e0:e0 + EG]).astype(bf16),
            "wvt": np.ascontiguousarray(wvt[:, e0:e0 + EG]).astype(bf16),
            "wot": np.ascontiguousarray(wot[e0:e0 + EG, :]).astype(bf16),
            "maskd": md.astype(bf16),
        })
    return in_maps


def gather_output(results):
    ys = [np.asarray(r["y"], np.float32) for r in results]
    out = np.empty((B, T, D), np.float32)
    for b in range(B):
        out[b] = sum(ys[b * NG:(b + 1) * NG])
    return out


last_run = {}


def kernel(x, Wq, Wk, Wv, Wo, _trace=False):
    from concourse.bass_utils import run_bass_kernel_spmd

    nc = get_nc()
    in_maps = prep_inputs(x, Wq, Wk, Wv, Wo)
    res = run_bass_kernel_spmd(
        nc, in_maps, core_ids=list(range(NCORES)), trace=_trace)
    last_run["results"] = res
    return gather_output(res.results)
